# revision 68
# baseline (speedup 1.0000x reference)
"""Single-head causal attention on 8 Trainium2 NeuronCores (Bass/Tile).

Problem: B=4, S=2048, D=E=1024 fp32.
  K = Xk @ WK; V = Xv @ WV; Q = Xq @ WQ
  att = softmax(causal(Q K^T / sqrt(S))) @ V;  returns (Q, att)

Sharding (uniform SPMD, per-core differences are data only):
  core c -> batch b = c // 2, kv parity s = c % 2.
  Each core handles ALL 2048 queries of its batch but only its parity
  half of the 16 kv chunks (abs chunk 2j+s for local j in [0,8)).  It
  emits flash-style partials (A = P~V sums, l = P~ row sums) and the
  host combines: att = (A0+A1)/(l0+l1).  This halves the K/V projection
  per core (a batch-pair split would duplicate it) at the cost of
  duplicating the cheaper Q projection.

Per-core kernel (matmul inputs bf16, psum f32, ap<=512):
  - Q phase: per 128-row q-tile: transpose Xq tile (PE), project
    Q = Xq Wq (ap512), emit Q rows (bf16), re-transpose to Q^T.
  - KV phase: per local chunk: transpose Xk/Xv rows, V = Xv Wv (ap512);
    per 512-col strip: K^T = Wk^T Xk^T (ap512).
  - Attention: per 256-col q-tile t, local chunks j<=t:
    S^T[k,q] = K^T chunk . Q^T (ap256), P~ = exp(scale*S^T) (bf16),
    causal mask (multiplicative, only at j==t, shift-invariant),
    l += P~^T 1 (ap2), A += P~^T V chunk (ap512).
  - Emission is software-pipelined so PE transposes/matmuls cover the
    DVE/scalar psum-drain latencies (PE p-state drops on any idle gap).
  - DMAs are batched into ~1MB transfers (the per-descriptor queue cost
    is ~1us; 175 small DMAs saturated the Sync queue).  Loads go on the
    Sync queue, stores on the otherwise-idle GpSimd queue.  Constants
    (identity/ones/mask) arrive via one DMA instead of gpsimd iota.
"""

import math
import sys

sys.path.insert(0, "/opt/trn_rl_repo")

import numpy as np  # noqa: E402
import ml_dtypes  # noqa: E402

import concourse.bass as bass  # noqa: E402
import concourse.tile as tile  # noqa: E402
from concourse import bacc, mybir  # noqa: E402
from concourse.bass_utils import run_bass_kernel_spmd  # noqa: E402

B, S, D, E = 4, 2048, 1024, 1024
NCORES = 8
SCALE = 1.0 / math.sqrt(float(S))
F32 = mybir.dt.float32
BF16 = mybir.dt.bfloat16
FP8 = mybir.dt.float8e4
NPB = np.dtype(ml_dtypes.bfloat16)
NPF8 = np.dtype(ml_dtypes.float8_e4m3)
DR = mybir.MatmulPerfMode.DoubleRow

NQT = S // 256  # 8 local q row-tiles (this core's half of the queries)
NKC = 8  # local kv chunks (parity half of 16)
NAT = S // 256  # 8 attention q-tiles (256 q cols each)
NCON = 128 + 2 + 256  # ident | ones | mask columns
QH = S // 2  # q rows owned per core


def build_nc(reps: int = 1):
    nc = bacc.Bacc("TRN2", target_bir_lowering=False, debug=False, num_devices=NCORES)

    xq_d = nc.dram_tensor("xq", [QH, D], BF16, kind="ExternalInput").ap()
    xk_d = nc.dram_tensor("xk", [NKC * 128, D], BF16, kind="ExternalInput").ap()
    xv_d = nc.dram_tensor("xv", [NKC * 128, D], BF16, kind="ExternalInput").ap()
    wq_d = nc.dram_tensor("wq", [D, E], BF16, kind="ExternalInput").ap()
    wk_d = nc.dram_tensor("wk", [D, E], BF16, kind="ExternalInput").ap()
    wv_d = nc.dram_tensor("wv", [D, E], BF16, kind="ExternalInput").ap()
    cn_d = nc.dram_tensor("consts", [128, NCON], BF16, kind="ExternalInput").ap()
    qo_d = nc.dram_tensor("q_out", [QH, E], BF16, kind="ExternalOutput").ap()
    ao_d = nc.dram_tensor("a_out", [S, E], BF16, kind="ExternalOutput").ap()
    lo_d = nc.dram_tensor("l_out", [NAT, 2, 128], F32, kind="ExternalOutput").ap()

    with tile.TileContext(nc) as tc:
        _emit(tc, reps, xq_d, xk_d, xv_d, wq_d, wk_d, wv_d, cn_d, qo_d, ao_d, lo_d)
    nc.compile()
    return nc


def _emit(tc, reps, xq_d, xk_d, xv_d, wq_d, wk_d, wv_d, cn_d, qo_d, ao_d, lo_d):
    nc = tc.nc
    with (
        tc.tile_pool(name="const", bufs=1) as cpool,
        tc.tile_pool(name="wp", bufs=3) as wpool,
        tc.tile_pool(name="big", bufs=1) as bigpool,
        tc.tile_pool(name="xq2", bufs=2) as xq2pool,
        tc.tile_pool(name="kv2", bufs=3) as kv2pool,
        tc.tile_pool(name="xt", bufs=3) as xtpool,
        tc.tile_pool(name="qrow", bufs=2) as qrowpool,
        tc.tile_pool(name="pt", bufs=3) as ptpool,
        tc.tile_pool(name="outp", bufs=2) as outpool,
        tc.tile_pool(name="dram", bufs=1, space="DRAM") as drampool,
    ):
        # ---- PE p-state pre-warm: ~5us of dummy matmuls with no DMA
        # deps run during the startup DMA wait, so the PE clock is fully
        # ramped (0.65->2.4 GHz takes ~3us of continuous work) when the
        # first real matmul issues.  Outputs are never read.
        warm = cpool.tile([128, 512], BF16)
        nc.vector.memset(warm[:], 1.0)
        with tc.tile_pool(name="warmp", bufs=1, space="PSUM") as warmpool:
            wps = warmpool.tile([128, 512], F32, tag="warm", name="warm")
            for _ in range(12):
                nc.tensor.matmul(wps[:], warm[:, :128], warm[:], start=True, stop=True)

        # ---- constants: ident first (first transpose needs only it) ---
        consts = cpool.tile([128, NCON], BF16)
        nc.sync.dma_start(consts[:, :128], cn_d[:, :128])
        nc.sync.dma_start(consts[:, 128:], cn_d[:, 128:])
        ident = consts[:, 0:128]
        ones = consts[:, 128:130]
        maskt = consts[:, 130:386]
        ls_big = cpool.tile([128, NAT, 2], F32)


        # weights: 2 DMAs each, on a caller-chosen trigger queue (each
        # trigger queue owns a DMA ring at ~110 GB/s; spread the 22MB of
        # traffic across sync/scalar/gpsimd/vector rings)
        def wload(w_d, nm, eng, dt=BF16):
            t = wpool.tile([128, D // 128, E], dt, tag="w", name=f"w{nm}")
            wr = w_d.rearrange("(c p) e -> p c e", p=128)
            for dh in range(2):
                eng.dma_start(t[:, 4 * dh : 4 * dh + 4], wr[:, 4 * dh : 4 * dh + 4])
            return t

        # big persistent tensors; the score path (Q^T, K^T, Xk^T) is fp8
        # for DoubleRow matmuls -- its rounding error is crushed by the
        # softmax's 1/sqrt(S) scale
        qt_big = bigpool.tile([128, E // 128, S], FP8, tag="qt", name="qt_big")
        qt_half = bigpool.tile([128, E // 128, QH], FP8, tag="qth", name="qt_half")
        # DRAM bounce buffers for the pair-wise Q^T AllGather
        qt_in = drampool.tile([E, QH], FP8, tag="qti", name="qt_in")
        qt_out = drampool.tile([2 * E, QH], FP8, tag="qto", name="qt_out")
        kt_big = bigpool.tile([128, E // 128, NKC * 128], FP8, tag="kt", name="kt")
        v_big = bigpool.tile([128, NKC, E], BF16, tag="v", name="v")
        xkt_big = bigpool.tile(
            [128, D // 128, NKC * 128], BF16, tag="xkt", name="xkt"
        )

        for _rep in range(reps):
            # PSUM pools: proj phases use trpool+prpool (4 banks); the
            # attention block uses its own 8 banks after these close.
            with (
                tc.tile_pool(name="trp", bufs=2, space="PSUM") as trpool,
                tc.tile_pool(name="prp", bufs=2, space="PSUM") as prpool,
            ):
                # ============ Q phase: project + transpose ==============
                def xq_load(u):
                    xl2 = xq2pool.tile([128, 2, D], BF16, tag="xl2", name=f"xq{u}")
                    src = xq_d[u * 256 : (u + 1) * 256, :].rearrange(
                        "(c p) d -> p c d", p=128
                    )
                    if u == 0:
                        # split the first load so tile 0 lands (and the PE
                        # starts transposing) half a DMA earlier
                        nc.sync.dma_start(xl2[:, 0], src[:, 0])
                        nc.sync.dma_start(xl2[:, 1], src[:, 1])
                    else:
                        nc.sync.dma_start(xl2[:], src)
                    return xl2

                def xq_tr(xl2, i):
                    xt = xtpool.tile(
                        [128, D // 128, 128], BF16, tag="xqt", name=f"xqt{i}"
                    )
                    for dc in range(D // 128):
                        pst = trpool.tile([128, 128], BF16, tag="tr", name="trq")
                        nc.tensor.transpose(
                            pst[:], xl2[:, i % 2, dc * 128 : (dc + 1) * 128], ident
                        )
                        nc.vector.tensor_copy(xt[:, dc], pst[:])
                    return xt

                def kv_load(g):
                    """Load xk/xv chunk pair g (chunks 2g, 2g+1)."""
                    xk2 = kv2pool.tile([128, 2, D], BF16, tag="xk2", name=f"xk{g}")
                    nc.sync.dma_start(
                        xk2[:],
                        xk_d[g * 256 : (g + 1) * 256, :].rearrange(
                            "(c p) d -> p c d", p=128
                        ),
                    )
                    xv2 = kv2pool.tile([128, 2, D], BF16, tag="xv2", name=f"xv{g}")
                    nc.sync.dma_start(
                        xv2[:],
                        xv_d[g * 256 : (g + 1) * 256, :].rearrange(
                            "(c p) d -> p c d", p=128
                        ),
                    )
                    return xk2, xv2

                xls = {0: xq_load(0)}
                xts = {0: xq_tr(xls[0], 0), 1: xq_tr(xls[0], 1)}
                wq_sb = wload(wq_d, "q", nc.sync)
                qrow2 = None
                for i in range(NQT):
                    u, c = divmod(i, 2)
                    xt = xts.pop(i)
                    if c == 0:
                        qrow2 = qrowpool.tile(
                            [128, 2, E], BF16, tag="qrow", name=f"q{u}"
                        )
                    for eh in range(2):
                        pr = prpool.tile([128, 512], F32, tag="pr", name="prq")
                        for dc in range(D // 128):
                            nc.tensor.matmul(
                                pr[:],
                                xt[:, dc],
                                wq_sb[:, dc, eh * 512 : (eh + 1) * 512],
                                start=(dc == 0),
                                stop=(dc == D // 128 - 1),
                            )
                        if eh == 0:
                            nc.vector.tensor_copy(
                                qrow2[:, c, :512], pr[:]
                            )
                        else:
                            nc.scalar.copy(qrow2[:, c, 512:], pr[:])
                    if c == 1:
                        nc.gpsimd.dma_start(
                            qo_d[u * 256 : (u + 1) * 256, :].rearrange(
                                "(c p) e -> p c e", p=128
                            ),
                            qrow2[:],
                        )
                    # prefetch xq transpose i+2 between Q matmuls and Q^T
                    # transposes: covers the psum->qrow drain latency on PE
                    # and keeps the xl DMA a pair ahead of its use
                    if i + 2 < NQT:
                        if (i + 2) % 2 == 0:
                            xls[u + 1] = xq_load(u + 1)
                            xls.pop(u - 1, None)
                        xts[i + 2] = xq_tr(xls[(i + 2) // 2], i + 2)
                    for ec in range(E // 128):
                        pst = trpool.tile([128, 128], BF16, tag="tr", name="trq2")
                        nc.tensor.transpose(
                            pst[:], qrow2[:, c, ec * 128 : (ec + 1) * 128], ident
                        )
                        nc.vector.tensor_copy(
                            qt_half[:, ec, i * 128 : (i + 1) * 128], pst[:]
                        )

                # ============ Q^T pair exchange =========================
                # Each core computed Q^T for its half of the queries; the
                # pair AllGathers the halves while the KV phase computes.
                nc.scalar.dma_start(
                    qt_in[:].rearrange("(c p) q -> p c q", p=128), qt_half[:]
                )
                nc.gpsimd.collective_compute(
                    "AllGather",
                    mybir.AluOpType.bypass,
                    replica_groups=[[0, 1], [2, 3], [4, 5], [6, 7]],
                    ins=[qt_in.opt()],
                    outs=[qt_out.opt()],
                )
                for s2 in range(2):
                    nc.gpsimd.dma_start(
                        qt_big[:, :, s2 * QH : (s2 + 1) * QH],
                        qt_out[s2 * E : (s2 + 1) * E, :].rearrange(
                            "(c p) q -> p c q", p=128
                        ),
                    )

                # ============ KV phase ==================================
                wk_sb = wload(wk_d, "k", nc.sync)
                wv_sb = wload(wv_d, "v", nc.sync)

                def kv_tr(kvl, j):
                    """Transpose xk chunk j into xkt_big; return xv^T chunk."""
                    xk2, xv2 = kvl
                    for dc in range(D // 128):
                        pst = trpool.tile([128, 128], BF16, tag="tr", name="trk")
                        nc.tensor.transpose(
                            pst[:], xk2[:, j % 2, dc * 128 : (dc + 1) * 128], ident
                        )
                        # the psum->sbuf copy converts bf16 -> fp8
                        nc.vector.tensor_copy(
                            xkt_big[:, dc, j * 128 : (j + 1) * 128], pst[:]
                        )
                    xvt = xtpool.tile(
                        [128, D // 128, 128], BF16, tag="xvt", name=f"xvt{j}"
                    )
                    for dc in range(D // 128):
                        pst = trpool.tile([128, 128], BF16, tag="tr", name="trv")
                        nc.tensor.transpose(
                            pst[:], xv2[:, j % 2, dc * 128 : (dc + 1) * 128], ident
                        )
                        nc.vector.tensor_copy(xvt[:, dc], pst[:])
                    return xvt

                def v_proj(j, xvt):
                    for eh in range(2):
                        pr = prpool.tile([128, 512], F32, tag="pr", name="prv")
                        for dc in range(D // 128):
                            nc.tensor.matmul(
                                pr[:],
                                xvt[:, dc],
                                wv_sb[:, dc, eh * 512 : (eh + 1) * 512],
                                start=(dc == 0),
                                stop=(dc == D // 128 - 1),
                            )
                        if eh == 0:
                            nc.vector.tensor_copy(v_big[:, j, :512], pr[:])
                        else:
                            nc.scalar.copy(v_big[:, j, 512:], pr[:])

                kvls = {0: kv_load(0)}
                xvts = {}
                for h in range(2):
                    for j in range(4 * h, 4 * h + 4):
                        if j % 2 == 0 and j // 2 + 1 < 4:
                            kvls[j // 2 + 1] = kv_load(j // 2 + 1)
                            kvls.pop(j // 2 - 1, None)
                        xvts[j] = kv_tr(kvls[j // 2], j)
                        if j - 1 in xvts:
                            v_proj(j - 1, xvts.pop(j - 1))
                    for ec in range(E // 128):
                        pr = prpool.tile([128, 512], F32, tag="pr", name="prk")
                        for dc in range(D // 128):
                            nc.tensor.matmul(
                                pr[:],
                                wk_sb[:, dc, ec * 128 : (ec + 1) * 128],
                                xkt_big[:, dc, h * 512 : (h + 1) * 512],
                                start=(dc == 0),
                                stop=(dc == D // 128 - 1),
                            )
                        if ec % 2 == 0:
                            nc.vector.tensor_copy(
                                kt_big[:, ec, h * 512 : (h + 1) * 512], pr[:]
                            )
                        else:
                            nc.scalar.copy(
                                kt_big[:, ec, h * 512 : (h + 1) * 512], pr[:]
                            )
                v_proj(7, xvts.pop(7))

            # ============ attention =====================================
            with (
                tc.tile_pool(name="spsp", bufs=2, space="PSUM") as spspool,
                tc.tile_pool(name="psap", bufs=4, space="PSUM") as psapool,
                tc.tile_pool(name="pslp", bufs=2, space="PSUM") as pslpool,
            ):
                # shard-0 tiles ascending (t=0 starts as soon as the
                # AllGather's first half lands), then shard-1 tiles
                # descending so the biggest tiles' stores drain under the
                # remaining compute instead of piling up at the end
                for t in [0, 1, 2, 3, 7, 6, 5, 4]:
                    a_ps = [
                        psapool.tile([128, 512], F32, tag="aps", name=f"a{t}_{q}")
                        for q in range(4)
                    ]
                    l_ps = [
                        pslpool.tile([128, 2], F32, tag="lps", name=f"l{t}_{st}")
                        for st in range(2)
                    ]

                    def st_mm(j, t=t):
                        sps = spspool.tile([128, 256], F32, tag="sps", name="sps")
                        for ec2 in range(E // 256):
                            nc.tensor.matmul(
                                sps[:],
                                kt_big[:, 2 * ec2 : 2 * ec2 + 2, j * 128 : (j + 1) * 128],
                                qt_big[:, 2 * ec2 : 2 * ec2 + 2, t * 256 : (t + 1) * 256],
                                start=(ec2 == 0),
                                stop=(ec2 == E // 256 - 1),
                                perf_mode=DR,
                            )
                        pt = ptpool.tile([128, 256], BF16, tag="pt", name="pt")
                        nc.scalar.activation(
                            pt[:], sps[:], mybir.ActivationFunctionType.Exp,
                            scale=SCALE,
                        )
                        if j == t:
                            nc.vector.tensor_mul(pt[:], pt[:], maskt)
                        return pt

                    pts = {0: st_mm(0)}
                    for j in range(t + 1):
                        if j + 1 <= t:
                            pts[j + 1] = st_mm(j + 1)
                        pt = pts.pop(j)
                        first, last = j == 0, j == t
                        for st in range(2):
                            nc.tensor.matmul(
                                l_ps[st][:],
                                pt[:, st * 128 : (st + 1) * 128],
                                ones,
                                start=first,
                                stop=last,
                            )
                            for eh in range(2):
                                nc.tensor.matmul(
                                    a_ps[2 * st + eh][:],
                                    pt[:, st * 128 : (st + 1) * 128],
                                    v_big[:, j, eh * 512 : (eh + 1) * 512],
                                    start=first,
                                    stop=last,
                                )
                    at2 = outpool.tile([128, 2, E], BF16, tag="at", name=f"at{t}")
                    for st in range(2):
                        if st == 0:
                            nc.vector.tensor_copy(at2[:, st, :512], a_ps[2 * st][:])
                            nc.scalar.copy(at2[:, st, 512:], a_ps[2 * st + 1][:])
                        else:
                            nc.scalar.copy(at2[:, st, :512], a_ps[2 * st][:])
                            nc.vector.tensor_copy(
                                at2[:, st, 512:], a_ps[2 * st + 1][:]
                            )
                        nc.vector.tensor_copy(
                            ls_big[:, t, st : st + 1], l_ps[st][:, 0:1]
                        )
                    if t == 4:
                        # final tile in the schedule: l store goes out
                        # before the last a store
                        nc.sync.dma_start(
                            lo_d.rearrange("t s p -> p t s"), ls_big[:]
                        )
                    # a stores split across the scalar and gpsimd rings so
                    # the two halves' wire times overlap (sync still owes
                    # loads early on; gpsimd is free after the Q phase)
                    nc.scalar.dma_start(
                        ao_d[t * 256 : t * 256 + 128, :], at2[:, 0]
                    )
                    nc.gpsimd.dma_start(
                        ao_d[t * 256 + 128 : t * 256 + 256, :], at2[:, 1]
                    )


def _mask(s: int) -> np.ndarray:
    kr = np.arange(128)[:, None]
    qr = np.arange(256)[None, :]
    return (s * 128 + kr <= qr).astype(NPB)


def _consts(s: int) -> np.ndarray:
    out = np.zeros((128, NCON), NPB)
    out[:, :128] = np.eye(128, dtype=np.float32).astype(NPB)
    out[:, 128:130] = 1.0
    out[:, 130:386] = _mask(s)
    return out


def make_core_inputs(xq_b, xk_b, xv_b, wq, wk, wv, s):
    """Per-core input dict. xq_b/xk_b/xv_b are the batch's [S, D] arrays."""
    return {
        "xq": np.ascontiguousarray(xq_b[s * QH : (s + 1) * QH].astype(NPB)),
        "xk": np.ascontiguousarray(
            xk_b.reshape(16, 128, D)[s::2].reshape(NKC * 128, D).astype(NPB)
        ),
        "xv": np.ascontiguousarray(
            xv_b.reshape(16, 128, D)[s::2].reshape(NKC * 128, D).astype(NPB)
        ),
        "wq": wq,
        "wk": wk,
        "wv": wv,
        "consts": _consts(s),
    }


_NC_CACHE = {}


def kernel(inputs_for_keys, inputs_for_values, inputs_for_queries, WK, WV, WQ):
    if "nc" not in _NC_CACHE:
        _NC_CACHE["nc"] = build_nc(1)
    nc = _NC_CACHE["nc"]

    xk = np.asarray(inputs_for_keys, np.float32)
    xv = np.asarray(inputs_for_values, np.float32)
    xq = np.asarray(inputs_for_queries, np.float32)
    wk = np.ascontiguousarray(np.asarray(WK, np.float32).astype(NPB))
    wv = np.ascontiguousarray(np.asarray(WV, np.float32).astype(NPB))
    wq = np.ascontiguousarray(np.asarray(WQ, np.float32).astype(NPB))

    in_maps = []
    for c in range(NCORES):
        b, s = c // 2, c % 2
        in_maps.append(make_core_inputs(xq[b], xk[b], xv[b], wq, wk, wv, s))
    res = run_bass_kernel_spmd(nc, in_maps, list(range(NCORES)))

    q_full = np.empty((B, S, E), np.float32)
    a_full = np.empty((B, S, E), np.float32)
    for b in range(B):
        r0 = res.results[2 * b]
        r1 = res.results[2 * b + 1]
        a = r0["a_out"].astype(np.float32) + r1["a_out"].astype(np.float32)
        l = (r0["l_out"] + r1["l_out"]).reshape(S)
        a_full[b] = a / l[:, None]
        q_full[b, :QH] = r0["q_out"].astype(np.float32)
        q_full[b, QH:] = r1["q_out"].astype(np.float32)
    return q_full, a_full
